# revision 9
# baseline (speedup 1.0000x reference)
"""Local (windowed) attention with rotary embeddings — Trainium2 Bass kernel.

Problem: nn_LocalAttention_46986942218547
  q,k,v: [8, 4, 4096, 64] f32, bin_attention_mask: [8, 4096] int32 (all ones)
  WINDOW=128, look_backward=1, causal. RoPE applied to q,k before attention.

Sharding: batch*heads (32 rows) split across 8 cores -> 4 rows/core.
Since H=4, core c gets exactly batch index c (all four heads), so the
per-batch bin mask needs no cross-core handling.

Precision: q,k,v are cast to bf16 on the host (halves HBM traffic); all
matmuls run bf16 with fp32 PSUM accumulation; exp/reciprocal/normalize in
fp32. Expected output error ~1e-3 relative to absmax.

Per-core pipeline (key window w serves query windows {w, w+1}):
  1. RoPE partial products in natural [pos, d] layout:
       u = [q|k]*cos,  t = swap([q|k])*ssin  (sign folded into ssin table),
       qkR = u + t      (DVE handles q columns, GPSIMD handles k columns)
  2. Two PE transposes per window (q half, k half) -> both land at PSUM
     partitions 0:64; copied to strips RQ (qRt) and LK (kRt), base 0.
  3. simT[j, i-pair] = matmul(lhsT=LK[w], rhs=RQ[w:w+2])  (bf16, N=256).
  4. pT = exp(simT/8) on ScalarE (softmax without max subtraction: logits
     bounded ~|7|), bf16 out; causal mask = bf16 multiply of the diagonal
     block by a lower-triangular 0/1 constant.
  5. acc[i, 0:65] = sum_w pT^T @ [v | 1]: one accumulating bf16 matmul per
     (query window, key window); column 64 is the softmax denominator.
  6. out = acc[:, 0:64] * (1/acc[:, 64]), one big f32 store per b-row.
"""

import sys

import numpy as np

for _p in ("/opt/trn_rl_repo",):
    if _p not in sys.path:
        sys.path.insert(0, _p)

import ml_dtypes

import concourse.bacc as bacc
import concourse.tile as tile
from concourse import mybir
from concourse.bass_utils import run_bass_kernel_spmd

F32 = mybir.dt.float32
BF16 = mybir.dt.bfloat16
BF16_NP = ml_dtypes.bfloat16

N_CORES = 8
B, H, SEQ, D = 8, 4, 4096, 64
WIN = 128
GRP = 4  # windows per batched group


def build_module(rb, n, apply_bin_mask, bcast_scale=True, repeat=None):
    """Build the per-core Bass module. rb: b-rows per core, n: seq length."""
    nw = n // WIN
    ng = nw // GRP
    assert nw % GRP == 0

    nc = bacc.Bacc("TRN2", target_bir_lowering=False, debug=False)

    q_d = nc.declare_dram_parameter("q", [rb, n, D], BF16, isOutput=False)
    k_d = nc.declare_dram_parameter("k", [rb, n, D], BF16, isOutput=False)
    v_d = nc.declare_dram_parameter("v", [rb, n, D], BF16, isOutput=False)
    cos_d = nc.declare_dram_parameter("costab", [n, D], BF16, isOutput=False)
    ssin_d = nc.declare_dram_parameter("ssintab", [n, D], BF16, isOutput=False)
    ident_d = nc.declare_dram_parameter("ident", [WIN, WIN], BF16, isOutput=False)
    lt_d = nc.declare_dram_parameter("ltmask", [WIN, GRP, WIN], BF16, isOutput=False)
    if apply_bin_mask:
        maskb_d = nc.declare_dram_parameter("maskb", [WIN, nw], F32, isOutput=False)
    out_d = nc.declare_dram_parameter("out", [rb, n, D], F32, isOutput=True)

    with tile.TileContext(nc) as tc:
        from contextlib import ExitStack

        with ExitStack() as ctx:
            consts = ctx.enter_context(tc.tile_pool(name="consts", bufs=1))
            strips = ctx.enter_context(tc.tile_pool(name="strips", bufs=2))
            tstrip = ctx.enter_context(tc.tile_pool(name="tstrip", bufs=2))
            quads = ctx.enter_context(tc.tile_pool(name="quads", bufs=3))
            outp = ctx.enter_context(tc.tile_pool(name="outp", bufs=2))
            ps_t = ctx.enter_context(tc.tile_pool(name="ps_t", bufs=2, space="PSUM"))
            ps_s = ctx.enter_context(tc.tile_pool(name="ps_s", bufs=2, space="PSUM"))
            ps_a = ctx.enter_context(tc.tile_pool(name="ps_a", bufs=2, space="PSUM"))

            cos_sb = consts.tile([WIN, nw, D], BF16)
            nc.sync.dma_start(cos_sb, cos_d.rearrange("(w p) d -> p w d", p=WIN))
            ssin_sb = consts.tile([WIN, nw, D], BF16)
            nc.sync.dma_start(ssin_sb, ssin_d.rearrange("(w p) d -> p w d", p=WIN))
            ident = consts.tile([WIN, WIN], BF16)
            nc.sync.dma_start(ident, ident_d[:])
            lt_sb = consts.tile([WIN, GRP, WIN], BF16)
            nc.sync.dma_start(lt_sb, lt_d[:])
            if apply_bin_mask:
                maskb_sb = consts.tile([WIN, nw], F32)
                nc.sync.dma_start(maskb_sb, maskb_d[:])

            rep_cm = (
                tc.For_i(
                    0, repeat, 1,
                    hint_engines=(
                        mybir.EngineType.PE,
                        mybir.EngineType.DVE,
                        mybir.EngineType.Activation,
                        mybir.EngineType.Pool,
                        mybir.EngineType.SP,
                    ),
                )
                if repeat
                else None
            )
            if rep_cm is not None:
                rep_cm.__enter__()
            for r in range(rb):
                q_s = strips.tile([WIN, nw, D], BF16, tag="qs")
                nc.sync.dma_start(q_s, q_d[r].rearrange("(w p) d -> p w d", p=WIN))
                k_s = strips.tile([WIN, nw, D], BF16, tag="ks")
                nc.sync.dma_start(k_s, k_d[r].rearrange("(w p) d -> p w d", p=WIN))
                # v strip carries an extra ones column per window for the
                # softmax-denominator trick.
                v_s = strips.tile([WIN, nw, D + 1], BF16, tag="vs")
                nc.sync.dma_start(
                    v_s[:, :, 0:D], v_d[r].rearrange("(w p) d -> p w d", p=WIN)
                )
                nc.gpsimd.memset(v_s[:, :, D : D + 1], 1.0)

                # Transposed strips, data at partitions 0:64.
                # RQ has one pad window so MM1's two-window rhs stays in bounds.
                rq_t = tstrip.tile([WIN, nw + 1, WIN], BF16, tag="rqt")
                nc.vector.memset(rq_t[0:64, nw, :], 0.0)
                lk_t = tstrip.tile([WIN, nw, WIN], BF16, tag="lkt")

                out_s = outp.tile([WIN, nw, D], F32, tag="outs")

                def fill(g):
                    """RoPE + transpose + copy-to-strips for windows of group g."""
                    ws = slice(g * GRP, (g + 1) * GRP)
                    # RoPE: qkR = [q|k]*cos + swap([q|k])*ssin  (bf16)
                    ut = quads.tile([WIN, GRP, 2, 2 * D], BF16, tag="ut")
                    qkr = quads.tile([WIN, GRP, 2 * D], BF16, tag="qkr")
                    # q columns on DVE
                    nc.vector.tensor_mul(ut[:, :, 0, 0:64], q_s[:, ws, :], cos_sb[:, ws, :])
                    nc.vector.tensor_mul(
                        ut[:, :, 1, 0:32], q_s[:, ws, 32:64], ssin_sb[:, ws, 0:32]
                    )
                    nc.vector.tensor_mul(
                        ut[:, :, 1, 32:64], q_s[:, ws, 0:32], ssin_sb[:, ws, 32:64]
                    )
                    nc.vector.tensor_add(
                        qkr[:, :, 0:64], ut[:, :, 0, 0:64], ut[:, :, 1, 0:64]
                    )
                    # k columns on GPSIMD
                    nc.gpsimd.tensor_mul(ut[:, :, 0, 64:128], k_s[:, ws, :], cos_sb[:, ws, :])
                    nc.gpsimd.tensor_mul(
                        ut[:, :, 1, 64:96], k_s[:, ws, 32:64], ssin_sb[:, ws, 0:32]
                    )
                    nc.gpsimd.tensor_mul(
                        ut[:, :, 1, 96:128], k_s[:, ws, 0:32], ssin_sb[:, ws, 32:64]
                    )
                    nc.gpsimd.tensor_add(
                        qkr[:, :, 64:128], ut[:, :, 0, 64:128], ut[:, :, 1, 64:128]
                    )

                    # PE transposes: q half and k half each -> [64,128] at base 0
                    tp = ps_t.tile([WIN, GRP, 2 * WIN], BF16, tag="tp")
                    for s in range(GRP):
                        nc.tensor.matmul(
                            tp[0:64, s, 0:WIN], qkr[:, s, 0:64], ident,
                            is_transpose=True, start=True, stop=True,
                        )
                        nc.tensor.matmul(
                            tp[0:64, s, WIN : 2 * WIN], qkr[:, s, 64:128], ident,
                            is_transpose=True, start=True, stop=True,
                        )
                    nc.vector.tensor_copy(rq_t[0:64, ws, :], tp[0:64, :, 0:WIN])
                    nc.scalar.copy(lk_t[0:64, ws, :], tp[0:64, :, WIN : 2 * WIN])

                def compute(g, acc_tiles):
                    """MM1/softmax/MM2/normalize for windows of group g.
                    Requires strips filled through window (g+1)*GRP (or pad)."""
                    w0 = g * GRP
                    ws = slice(w0, w0 + GRP)
                    # MM1: simT[j, i-pair], bf16, N=256
                    st = ps_s.tile([WIN, GRP, 2 * WIN], F32, tag="st")
                    for s in range(GRP):
                        w = w0 + s
                        rhs = rq_t[0:64, w : w + 2, :].rearrange("p a b -> p (a b)")
                        nc.tensor.matmul(
                            st[:, s, :], lk_t[0:64, w, :], rhs, start=True, stop=True
                        )

                    # exp(sim/8); bf16 out. Optional per-key bin-mask bias.
                    pt = quads.tile([WIN, GRP, 2 * WIN], BF16, tag="pt")
                    if apply_bin_mask:
                        for s in range(GRP):
                            w = w0 + s
                            nc.scalar.activation(
                                pt[:, s, :], st[:, s, :],
                                mybir.ActivationFunctionType.Exp,
                                bias=maskb_sb[:, w : w + 1], scale=0.125,
                            )
                    else:
                        nc.scalar.activation(
                            pt, st, mybir.ActivationFunctionType.Exp, scale=0.125
                        )

                    # causal mask on the diagonal-block halves (GPSIMD)
                    nc.gpsimd.tensor_mul(pt[:, :, 0:WIN], pt[:, :, 0:WIN], lt_sb)

                    # MM2: accumulate attn@[v|1] per query window.
                    if g not in acc_tiles:
                        acc_tiles[g] = ps_a.tile(
                            [WIN, GRP, WIN], F32, tag="acc", name=f"acc_{r}_{g}"
                        )
                    acc = acc_tiles.pop(g)
                    if g + 1 < ng and g + 1 not in acc_tiles:
                        acc_tiles[g + 1] = ps_a.tile(
                            [WIN, GRP, WIN], F32, tag="acc", name=f"acc_{r}_{g + 1}"
                        )
                    for s in range(GRP):
                        w = w0 + s
                        nc.tensor.matmul(
                            acc[:, s, 0 : D + 1], pt[:, s, 0:WIN], v_s[:, w, :],
                            start=(w == 0), stop=True, skip_group_check=True,
                        )
                        if w + 1 < nw:
                            tgt = (
                                acc[:, s + 1, 0 : D + 1]
                                if s + 1 < GRP
                                else acc_tiles[g + 1][:, 0, 0 : D + 1]
                            )
                            nc.tensor.matmul(
                                tgt, pt[:, s, WIN : 2 * WIN], v_s[:, w, :],
                                start=True, stop=False, skip_group_check=True,
                            )

                    # normalize: out = acc[:, :64] / acc[:, 64]
                    rinv = quads.tile([WIN, GRP, 1], F32, tag="rinv")
                    nc.vector.reciprocal(rinv, acc[:, :, D : D + 1])
                    if bcast_scale:
                        import concourse.bass as bass

                        rb_ap = rinv[:, :, 0]  # [128, GRP]
                        rbc = bass.AP(
                            tensor=rb_ap.tensor,
                            offset=rb_ap.offset,
                            ap=list(rb_ap.ap) + [[0, D]],
                        )
                        nc.vector.tensor_mul(out_s[:, ws, :], acc[:, :, 0:D], rbc)
                    else:
                        for s in range(GRP):
                            nc.scalar.mul(
                                out_s[:, w0 + s, :], acc[:, s, 0:D], rinv[:, s, :]
                            )

                # compute(g) reads the first window of group g+1, so run it
                # one fill behind.
                acc_tiles = {}
                for g in range(ng):
                    fill(g)
                    if g >= 1:
                        compute(g - 1, acc_tiles)
                compute(ng - 1, acc_tiles)

                nc.scalar.dma_start(
                    out_d[r].rearrange("(w p) d -> p w d", p=WIN), out_s
                )
            if rep_cm is not None:
                rep_cm.__exit__(None, None, None)

    nc.compile()
    return nc


def host_tables(n):
    inv_freq = (1.0 / (10000.0 ** (np.arange(0, D, 2, dtype=np.float32) / D))).astype(
        np.float32
    )
    t = np.arange(n, dtype=np.float32)
    freqs = np.einsum("i,j->ij", t, inv_freq).astype(np.float32)  # [n, 32]
    cos = np.cos(np.concatenate([freqs, freqs], axis=-1)).astype(BF16_NP)  # [n, 64]
    sinf = np.sin(freqs).astype(np.float32)  # [n, 32]
    ssin = np.concatenate([-sinf, sinf], axis=-1).astype(BF16_NP)  # [n, 64]
    ident = np.eye(WIN, dtype=np.float32).astype(BF16_NP)
    lt = np.triu(np.ones((WIN, WIN), dtype=np.float32))  # lt[j, i] = 1 iff i >= j
    lt = np.broadcast_to(lt[:, None, :], (WIN, GRP, WIN)).astype(BF16_NP)
    return cos, ssin, ident, np.ascontiguousarray(lt)


_MODULE_CACHE = {}
_last_in_maps = None


def _get_module(key, *args, **kwargs):
    if key not in _MODULE_CACHE:
        _MODULE_CACHE[key] = build_module(*args, **kwargs)
    return _MODULE_CACHE[key]


def kernel(q, k, v, bin_attention_mask):
    Bq, Hq, n, d = q.shape
    assert (Bq, Hq, n, d) == (B, H, SEQ, D), (q.shape,)
    rb = (Bq * Hq) // N_CORES

    qf = np.asarray(q).reshape(Bq * Hq, n, d).astype(BF16_NP)
    kf = np.asarray(k).reshape(Bq * Hq, n, d).astype(BF16_NP)
    vf = np.asarray(v).reshape(Bq * Hq, n, d).astype(BF16_NP)

    mask = np.asarray(bin_attention_mask)
    apply_bin_mask = not bool(mask.all())

    cos, ssin, ident, lt = host_tables(n)

    nc = _get_module(("full", rb, n, apply_bin_mask), rb, n, apply_bin_mask)

    in_maps = []
    for c in range(N_CORES):
        m = {
            "q": np.ascontiguousarray(qf[c * rb : (c + 1) * rb]),
            "k": np.ascontiguousarray(kf[c * rb : (c + 1) * rb]),
            "v": np.ascontiguousarray(vf[c * rb : (c + 1) * rb]),
            "costab": cos,
            "ssintab": ssin,
            "ident": ident,
            "ltmask": lt,
        }
        if apply_bin_mask:
            bidx = (c * rb) // H
            mb = np.where(mask[bidx].astype(bool), 0.0, -1e9).astype(np.float32)
            m["maskb"] = np.ascontiguousarray(mb.reshape(n // WIN, WIN).T)
        in_maps.append(m)

    global _last_in_maps
    _last_in_maps = in_maps
    res = run_bass_kernel_spmd(nc, in_maps, core_ids=list(range(N_CORES)))
    outs = [res.results[c]["out"] for c in range(N_CORES)]
    out = np.concatenate(outs, axis=0).reshape(Bq, Hq, n, d).astype(np.float32)
    return out


# revision 12
# speedup vs baseline: 1.3674x; 1.3674x over previous
"""Local (windowed) attention with rotary embeddings — Trainium2 Bass kernel.

Problem: nn_LocalAttention_46986942218547
  q,k,v: [8, 4, 4096, 64] f32, bin_attention_mask: [8, 4096] int32 (all ones)
  WINDOW=128, look_backward=1, causal. RoPE applied to q,k before attention.

Sharding: batch*heads (32 rows) split across 8 cores -> 4 rows/core.
Since H=4, core c gets exactly batch index c (all four heads), so the
per-batch bin mask needs no cross-core handling.

Precision: q,k,v are cast to bf16 on the host (halves HBM traffic); all
matmuls run bf16 with fp32 PSUM accumulation; exp/reciprocal/normalize in
fp32. Measured output error ~4e-3 relative to absmax(expected).

Per-core pipeline (key window w serves query windows {w, w+1}):
  1. fill(g):  RoPE partial products in natural [pos, d] layout
       u = [q|k]*cos,  t = swap([q|k])*ssin  (sign folded into ssin table),
       qkR = u + t; two PE transposes per window (q half, k half) land at
       PSUM partitions 0:64 and are copied to strips RQ (qRt) / LK (kRt).
  2. compute(g): simT[j, i-pair] = matmul(lhsT=LK[w], rhs=RQ[w:w+2]) (bf16,
     N=256); pT = exp(simT/8) on ScalarE (no max subtraction: logits are
     bounded ~|7|), bf16 out; causal mask = bf16 multiply of the diagonal
     block by a lower-triangular 0/1 constant; acc[i,0:65] accumulates
     pT^T @ [v | 1] (column 64 = softmax denominator); out = acc[:, :64] *
     (1/acc[:, 64]).
  Emission order is a 2-group software pipeline — compute(g-2) is emitted
  BEFORE fill(g) so each engine's instruction stream has its ready work
  first (engine streams execute in emission order; putting blocked fill
  work ahead of ready compute work serializes the whole kernel).
"""

import sys

import numpy as np

for _p in ("/opt/trn_rl_repo",):
    if _p not in sys.path:
        sys.path.insert(0, _p)

import ml_dtypes

import concourse.bacc as bacc
import concourse.tile as tile
from concourse import mybir
from concourse.bass_utils import run_bass_kernel_spmd

F32 = mybir.dt.float32
BF16 = mybir.dt.bfloat16
BF16_NP = ml_dtypes.bfloat16

N_CORES = 8
B, H, SEQ, D = 8, 4, 4096, 64
WIN = 128
GRP = 4  # windows per batched group


def build_module(
    rb,
    n,
    apply_bin_mask,
    bcast_scale=True,
    repeat=None,
    mask_engine="vector",
    krope_split=True,
):
    """Build the per-core Bass module. rb: b-rows per core, n: seq length."""
    nw = n // WIN
    ng = nw // GRP
    assert nw % GRP == 0

    nc = bacc.Bacc("TRN2", target_bir_lowering=False, debug=False)

    q_d = nc.declare_dram_parameter("q", [rb, n, D], BF16, isOutput=False)
    k_d = nc.declare_dram_parameter("k", [rb, n, D], BF16, isOutput=False)
    v_d = nc.declare_dram_parameter("v", [rb, n, D], BF16, isOutput=False)
    cos_d = nc.declare_dram_parameter("costab", [n, D], BF16, isOutput=False)
    ssin_d = nc.declare_dram_parameter("ssintab", [n, D], BF16, isOutput=False)
    ident_d = nc.declare_dram_parameter("ident", [WIN, WIN], BF16, isOutput=False)
    lt_d = nc.declare_dram_parameter("ltmask", [WIN, GRP, WIN], BF16, isOutput=False)
    if apply_bin_mask:
        maskb_d = nc.declare_dram_parameter("maskb", [WIN, nw], F32, isOutput=False)
    out_d = nc.declare_dram_parameter("out", [rb, n, D], F32, isOutput=True)

    with tile.TileContext(nc) as tc:
        from contextlib import ExitStack

        with ExitStack() as ctx:
            consts = ctx.enter_context(tc.tile_pool(name="consts", bufs=1))
            strips = ctx.enter_context(tc.tile_pool(name="strips", bufs=2))
            tstrip = ctx.enter_context(tc.tile_pool(name="tstrip", bufs=2))
            quads = ctx.enter_context(tc.tile_pool(name="quads", bufs=3))
            outp = ctx.enter_context(tc.tile_pool(name="outp", bufs=2))
            ps_t = ctx.enter_context(tc.tile_pool(name="ps_t", bufs=2, space="PSUM"))
            ps_s = ctx.enter_context(tc.tile_pool(name="ps_s", bufs=2, space="PSUM"))
            ps_a = ctx.enter_context(tc.tile_pool(name="ps_a", bufs=2, space="PSUM"))

            cos_sb = consts.tile([WIN, nw, D], BF16)
            nc.sync.dma_start(cos_sb, cos_d.rearrange("(w p) d -> p w d", p=WIN))
            ssin_sb = consts.tile([WIN, nw, D], BF16)
            nc.sync.dma_start(ssin_sb, ssin_d.rearrange("(w p) d -> p w d", p=WIN))
            ident = consts.tile([WIN, WIN], BF16)
            nc.sync.dma_start(ident, ident_d[:])
            lt_sb = consts.tile([WIN, GRP, WIN], BF16)
            nc.sync.dma_start(lt_sb, lt_d[:])
            if apply_bin_mask:
                maskb_sb = consts.tile([WIN, nw], F32)
                nc.sync.dma_start(maskb_sb, maskb_d[:])

            mask_eng = nc.vector if mask_engine == "vector" else nc.gpsimd

            rep_cm = (
                tc.For_i(
                    0, repeat, 1,
                    hint_engines=(
                        mybir.EngineType.PE,
                        mybir.EngineType.DVE,
                        mybir.EngineType.Activation,
                        mybir.EngineType.Pool,
                        mybir.EngineType.SP,
                    ),
                )
                if repeat
                else None
            )
            if rep_cm is not None:
                rep_cm.__enter__()
            for r in range(rb):
                q_s = strips.tile([WIN, nw, D], BF16, tag="qs")
                nc.sync.dma_start(q_s, q_d[r].rearrange("(w p) d -> p w d", p=WIN))
                k_s = strips.tile([WIN, nw, D], BF16, tag="ks")
                nc.sync.dma_start(k_s, k_d[r].rearrange("(w p) d -> p w d", p=WIN))
                # v strip carries an extra ones column per window for the
                # softmax-denominator trick.
                v_s = strips.tile([WIN, nw, D + 1], BF16, tag="vs")
                nc.sync.dma_start(
                    v_s[:, :, 0:D], v_d[r].rearrange("(w p) d -> p w d", p=WIN)
                )
                nc.gpsimd.memset(v_s[:, :, D : D + 1], 1.0)

                # Transposed strips, data at partitions 0:64.
                # RQ has one pad window so MM1's two-window rhs stays in bounds.
                rq_t = tstrip.tile([WIN, nw + 1, WIN], BF16, tag="rqt")
                nc.vector.memset(rq_t[0:64, nw, :], 0.0)
                lk_t = tstrip.tile([WIN, nw, WIN], BF16, tag="lkt")

                out_s = outp.tile([WIN, nw, D], F32, tag="outs")

                def fill(g):
                    """RoPE + transpose + copy-to-strips for windows of group g."""
                    ws = slice(g * GRP, (g + 1) * GRP)
                    ut = quads.tile([WIN, GRP, 2, 2 * D], BF16, tag="ut")
                    qkr = quads.tile([WIN, GRP, 2 * D], BF16, tag="qkr")
                    # q columns on DVE
                    nc.vector.tensor_mul(ut[:, :, 0, 0:64], q_s[:, ws, :], cos_sb[:, ws, :])
                    nc.vector.tensor_mul(
                        ut[:, :, 1, 0:32], q_s[:, ws, 32:64], ssin_sb[:, ws, 0:32]
                    )
                    nc.vector.tensor_mul(
                        ut[:, :, 1, 32:64], q_s[:, ws, 0:32], ssin_sb[:, ws, 32:64]
                    )
                    nc.vector.tensor_add(
                        qkr[:, :, 0:64], ut[:, :, 0, 0:64], ut[:, :, 1, 0:64]
                    )
                    # k columns on GPSIMD (optionally split with DVE)
                    kmul1 = nc.vector if krope_split else nc.gpsimd
                    nc.gpsimd.tensor_mul(ut[:, :, 0, 64:128], k_s[:, ws, :], cos_sb[:, ws, :])
                    kmul1.tensor_mul(
                        ut[:, :, 1, 64:96], k_s[:, ws, 32:64], ssin_sb[:, ws, 0:32]
                    )
                    kmul1.tensor_mul(
                        ut[:, :, 1, 96:128], k_s[:, ws, 0:32], ssin_sb[:, ws, 32:64]
                    )
                    nc.gpsimd.tensor_add(
                        qkr[:, :, 64:128], ut[:, :, 0, 64:128], ut[:, :, 1, 64:128]
                    )

                    # PE transposes: q half and k half each -> [64,128] at base 0
                    tp = ps_t.tile([WIN, GRP, 2 * WIN], BF16, tag="tp")
                    for s in range(GRP):
                        nc.tensor.matmul(
                            tp[0:64, s, 0:WIN], qkr[:, s, 0:64], ident,
                            is_transpose=True, start=True, stop=True,
                        )
                        nc.tensor.matmul(
                            tp[0:64, s, WIN : 2 * WIN], qkr[:, s, 64:128], ident,
                            is_transpose=True, start=True, stop=True,
                        )
                    nc.vector.tensor_copy(rq_t[0:64, ws, :], tp[0:64, :, 0:WIN])
                    nc.scalar.copy(lk_t[0:64, ws, :], tp[0:64, :, WIN : 2 * WIN])

                def compute(g, acc_tiles):
                    """MM1/softmax/MM2/normalize for windows of group g.
                    Requires strips filled through window (g+1)*GRP (or pad)."""
                    w0 = g * GRP
                    ws = slice(w0, w0 + GRP)
                    # MM1: simT[j, i-pair], bf16, N=256
                    st = ps_s.tile([WIN, GRP, 2 * WIN], F32, tag="st")
                    for s in range(GRP):
                        w = w0 + s
                        rhs = rq_t[0:64, w : w + 2, :].rearrange("p a b -> p (a b)")
                        nc.tensor.matmul(
                            st[:, s, :], lk_t[0:64, w, :], rhs, start=True, stop=True
                        )

                    # exp(sim/8); bf16 out. Optional per-key bin-mask bias.
                    pt = quads.tile([WIN, GRP, 2 * WIN], BF16, tag="pt")
                    if apply_bin_mask:
                        for s in range(GRP):
                            w = w0 + s
                            nc.scalar.activation(
                                pt[:, s, :], st[:, s, :],
                                mybir.ActivationFunctionType.Exp,
                                bias=maskb_sb[:, w : w + 1], scale=0.125,
                            )
                    else:
                        nc.scalar.activation(
                            pt, st, mybir.ActivationFunctionType.Exp, scale=0.125
                        )

                    # causal mask on the diagonal-block halves
                    mask_eng.tensor_mul(pt[:, :, 0:WIN], pt[:, :, 0:WIN], lt_sb)

                    # MM2: accumulate attn@[v|1] per query window.
                    if g not in acc_tiles:
                        acc_tiles[g] = ps_a.tile(
                            [WIN, GRP, WIN], F32, tag="acc", name=f"acc_{r}_{g}"
                        )
                    acc = acc_tiles.pop(g)
                    if g + 1 < ng and g + 1 not in acc_tiles:
                        acc_tiles[g + 1] = ps_a.tile(
                            [WIN, GRP, WIN], F32, tag="acc", name=f"acc_{r}_{g + 1}"
                        )
                    for s in range(GRP):
                        w = w0 + s
                        nc.tensor.matmul(
                            acc[:, s, 0 : D + 1], pt[:, s, 0:WIN], v_s[:, w, :],
                            start=(w == 0), stop=True, skip_group_check=True,
                        )
                        if w + 1 < nw:
                            tgt = (
                                acc[:, s + 1, 0 : D + 1]
                                if s + 1 < GRP
                                else acc_tiles[g + 1][:, 0, 0 : D + 1]
                            )
                            nc.tensor.matmul(
                                tgt, pt[:, s, WIN : 2 * WIN], v_s[:, w, :],
                                start=True, stop=False, skip_group_check=True,
                            )

                    # normalize: out = acc[:, :64] / acc[:, 64]
                    rinv = quads.tile([WIN, GRP, 1], F32, tag="rinv")
                    nc.vector.reciprocal(rinv, acc[:, :, D : D + 1])
                    if bcast_scale:
                        import concourse.bass as bass

                        rb_ap = rinv[:, :, 0]  # [128, GRP]
                        rbc = bass.AP(
                            tensor=rb_ap.tensor,
                            offset=rb_ap.offset,
                            ap=list(rb_ap.ap) + [[0, D]],
                        )
                        nc.vector.tensor_mul(out_s[:, ws, :], acc[:, :, 0:D], rbc)
                    else:
                        for s in range(GRP):
                            nc.scalar.mul(
                                out_s[:, w0 + s, :], acc[:, s, 0:D], rinv[:, s, :]
                            )

                # 2-group software pipeline: compute(g-2) before fill(g) so
                # every engine sees its ready work first.
                acc_tiles = {}
                for gi in range(ng + 2):
                    if gi >= 2:
                        compute(gi - 2, acc_tiles)
                    if gi < ng:
                        fill(gi)

                nc.scalar.dma_start(
                    out_d[r].rearrange("(w p) d -> p w d", p=WIN), out_s
                )
            if rep_cm is not None:
                rep_cm.__exit__(None, None, None)

    nc.compile()
    return nc


def host_tables(n):
    inv_freq = (1.0 / (10000.0 ** (np.arange(0, D, 2, dtype=np.float32) / D))).astype(
        np.float32
    )
    t = np.arange(n, dtype=np.float32)
    freqs = np.einsum("i,j->ij", t, inv_freq).astype(np.float32)  # [n, 32]
    cos = np.cos(np.concatenate([freqs, freqs], axis=-1)).astype(BF16_NP)  # [n, 64]
    sinf = np.sin(freqs).astype(np.float32)  # [n, 32]
    ssin = np.concatenate([-sinf, sinf], axis=-1).astype(BF16_NP)  # [n, 64]
    ident = np.eye(WIN, dtype=np.float32).astype(BF16_NP)
    lt = np.triu(np.ones((WIN, WIN), dtype=np.float32))  # lt[j, i] = 1 iff i >= j
    lt = np.broadcast_to(lt[:, None, :], (WIN, GRP, WIN)).astype(BF16_NP)
    return cos, ssin, ident, np.ascontiguousarray(lt)


_MODULE_CACHE = {}
_last_in_maps = None


def _get_module(key, *args, **kwargs):
    if key not in _MODULE_CACHE:
        _MODULE_CACHE[key] = build_module(*args, **kwargs)
    return _MODULE_CACHE[key]


def kernel(q, k, v, bin_attention_mask):
    Bq, Hq, n, d = q.shape
    assert (Bq, Hq, n, d) == (B, H, SEQ, D), (q.shape,)
    rb = (Bq * Hq) // N_CORES

    qf = np.asarray(q).reshape(Bq * Hq, n, d).astype(BF16_NP)
    kf = np.asarray(k).reshape(Bq * Hq, n, d).astype(BF16_NP)
    vf = np.asarray(v).reshape(Bq * Hq, n, d).astype(BF16_NP)

    mask = np.asarray(bin_attention_mask)
    apply_bin_mask = not bool(mask.all())

    cos, ssin, ident, lt = host_tables(n)

    nc = _get_module(("full", rb, n, apply_bin_mask), rb, n, apply_bin_mask)

    in_maps = []
    for c in range(N_CORES):
        m = {
            "q": np.ascontiguousarray(qf[c * rb : (c + 1) * rb]),
            "k": np.ascontiguousarray(kf[c * rb : (c + 1) * rb]),
            "v": np.ascontiguousarray(vf[c * rb : (c + 1) * rb]),
            "costab": cos,
            "ssintab": ssin,
            "ident": ident,
            "ltmask": lt,
        }
        if apply_bin_mask:
            bidx = (c * rb) // H
            mb = np.where(mask[bidx].astype(bool), 0.0, -1e9).astype(np.float32)
            m["maskb"] = np.ascontiguousarray(mb.reshape(n // WIN, WIN).T)
        in_maps.append(m)

    global _last_in_maps
    _last_in_maps = in_maps
    res = run_bass_kernel_spmd(nc, in_maps, core_ids=list(range(N_CORES)))
    outs = [res.results[c]["out"] for c in range(N_CORES)]
    out = np.concatenate(outs, axis=0).reshape(Bq, Hq, n, d).astype(np.float32)
    return out
